# revision 12
# baseline (speedup 1.0000x reference)
"""NextVLAD Trainium2 kernel v7 — 8-way data-parallel over batch (1 sample/core).

Host prep: x is token-normalized on host (exact), so every device scale is a
compile-time constant; W3 = (W_gk@W_inp)^T*SW3 folds the fc projection for
the softmax path; WgT = (W_g@W_inp)^T*SW3 feeds a flipped gate matmul;
W1 = W_inp^T*SW1 feeds yT. Final L2 norms and the +b_inp contribution are
applied on host (the device exports per-group sums Sg via one-hot columns
in the vlad matmul).

v7 layout/schedule:
- every input host-packed to its exact [128, X] SBUF image -> each DMA is
  full-partition-row contiguous (big packets); one ring (sync), strict
  priority order: xa -> w3blk0 -> xb -> w3blk1 -> wgT -> cf -> w1a -> w1b.
- gate logits via a FLIPPED matmul (out [8, M]): one 8-mm chain instead of
  16 LDW-bound [*,8] matmuls; sigmoid computed in [8, M] layout, then 4 PE
  transposes (identity in cf) give sg[m, g] for the wf weights.
- yT = ry*YTS on the Act engine (Copy+scale from PSUM); wf = ex*sg*ise on
  DVE, all 32 tiles emitted right after the softmax denominators.

Per-core dataflow (sample b; M=512 tokens, N=1024, EN=2048, G=8, K=128, D=256):
  z[m,h512]  = x8^T W3-blk (fp8 DR)     ex = exp(z/16384)        bf16
  zgT[g,m]   = wgT^T x8 (fp8 DR)        egT = exp(zgT/16384)     f32
                                        sgT = egT/(egT+e^{-bg'}) DVE
                                        sg[m,g] = transpose(sgT) PE
  se[h]      = ones^T ex (bf16 mm)      ise = recip_approx(se)   f32
  rawY[m,e]  = x8^T W1-chunks (fp8 DR)  yT = rawY/128 (Act Copy) bf16
  wf[m,gk]   = ex * sg * ise (DVE)                               bf16
  vd[k,266]  = sum_{g,m} wf_g^T [yT_g | 1 1 | e_g]
               (col 256 = S[k], cols 258+g = Sg[k,g])
  out[k,:256]= vd - S*cent ; out[k,256:264] = Sg  (host: +Sg@binp, l2norm)
"""
import os
import numpy as np

N = 1024          # feature size
EN = 2048         # expanded features
G = 8             # groups
KC = 128          # clusters
D = 256           # per-group cluster dim
GK = G * KC       # 1024
BW = D + 10       # yT group block: 256 data | 2 ones | 8 one-hot = 266
M = 512           # tokens per sample
MT = 4            # m-tiles of 128
CFW = 280         # cf width: 256 cent | 8 spare | I8 (264:272) | ebgT (272)
SX = 8.0
SW1 = 16.0
SW3 = 2048.0
EXS = 1.0 / (SX * SW3)    # 1/16384
YTS = 1.0 / (SX * SW1)    # 1/128

_cache = {}


def _build_nc():
    import concourse.bacc as bacc
    import concourse.tile as tile
    from concourse import mybir

    f32 = mybir.dt.float32
    bf16 = mybir.dt.bfloat16
    fp8 = mybir.dt.float8e4
    Alu = mybir.AluOpType
    Act = mybir.ActivationFunctionType
    DR = mybir.MatmulPerfMode.DoubleRow

    nc = bacc.Bacc("TRN2", target_bir_lowering=False)
    # host-packed: row p, col (cs*W + j) holds source row cs*128+p, col j
    xa_d = nc.dram_tensor("xa", [128, 8 * 256], fp8, kind="ExternalInput")
    xb_d = nc.dram_tensor("xb", [128, 8 * 256], fp8, kind="ExternalInput")
    w3h0_d = nc.dram_tensor("w3h0", [128, 8 * 512], fp8, kind="ExternalInput")
    w3h1_d = nc.dram_tensor("w3h1", [128, 8 * 512], fp8, kind="ExternalInput")
    wgt_d = nc.dram_tensor("wgt", [128, 8 * 16], fp8, kind="ExternalInput")
    w1a_d = nc.dram_tensor("w1a", [128, 8 * 1024], fp8, kind="ExternalInput")
    w1b_d = nc.dram_tensor("w1b", [128, 8 * 1024], fp8, kind="ExternalInput")
    cf_d = nc.dram_tensor("cf", [128, CFW], f32, kind="ExternalInput")
    out_d = nc.dram_tensor("out", [KC, D + G], f32, kind="ExternalOutput")

    with tile.TileContext(nc) as tc:
        with tc.tile_pool(name="const", bufs=1) as constp, \
             tc.tile_pool(name="data", bufs=1) as datap, \
             tc.tile_pool(name="work", bufs=1) as workp, \
             tc.tile_pool(name="ps", bufs=1, space="PSUM") as ps:

            # ---------------- tiles ----------------
            cf_t = constp.tile([128, CFW], f32)
            crb_t = constp.tile([128, 128], bf16)
            centn_t = cf_t[:, 0:D]
            i8_t = cf_t[0:8, 264:272]
            ebgT_t = cf_t[0:8, 272:273]
            xa_t = datap.tile([128, 8 * 256], fp8)
            xav = xa_t.rearrange("p (cs m) -> p cs m", m=256)
            xb_t = datap.tile([128, 8 * 256], fp8)
            xbv = xb_t.rearrange("p (cs m) -> p cs m", m=256)
            w3h0_t = datap.tile([128, 8 * 512], fp8)
            w3h0v = w3h0_t.rearrange("p (cs j) -> p cs j", j=512)
            w3h1_t = datap.tile([128, 8 * 512], fp8)
            w3h1v = w3h1_t.rearrange("p (cs j) -> p cs j", j=512)
            wgt_t = datap.tile([128, 8 * 16], fp8)
            wgtv = wgt_t.rearrange("p (cs j) -> p cs j", j=16)
            w1a_t = datap.tile([128, 8 * 1024], fp8)
            w1av = w1a_t.rearrange("p (cs j) -> p cs j", j=1024)
            w1b_t = datap.tile([128, 8 * 1024], fp8)
            w1bv = w1b_t.rearrange("p (cs j) -> p cs j", j=1024)

            def xop(m, c):
                """lhsT for m-tile m, chunk c: [128, 2, 128]."""
                v = xav if m < 2 else xbv
                ms = (m % 2) * 128
                return v[:, 2 * c:2 * c + 2, ms:ms + 128]

            # ------------- input DMA: one ring (sync), strict priority order ------
            nc.sync.dma_start(out=xa_t[:], in_=xa_d[:])
            nc.sync.dma_start(out=w3h0_t[:], in_=w3h0_d[:])
            nc.sync.dma_start(out=xb_t[:], in_=xb_d[:])
            nc.sync.dma_start(out=w3h1_t[:], in_=w3h1_d[:])
            nc.sync.dma_start(out=wgt_t[:], in_=wgt_d[:])
            nc.sync.dma_start(out=cf_t[:], in_=cf_d[:])
            nc.sync.dma_start(out=w1a_t[:], in_=w1a_d[:])
            nc.sync.dma_start(out=w1b_t[:], in_=w1b_d[:])

            # persistent work tiles
            ex_t = [workp.tile([128, GK], bf16, name=f"ex{m}") for m in range(MT)]
            wf_t = [workp.tile([128, GK], bf16, name=f"wf{m}") for m in range(MT)]
            yT_t = [[workp.tile([128, 2 * BW], bf16, name=f"yT{e}_{m}")
                     for m in range(MT)] for e in range(4)]
            ise_t = workp.tile([128, GK], f32)
            sg_t = workp.tile([128, 4 * G], f32)
            egT_t = workp.tile([128, M], f32)
            sgT_t = workp.tile([128, M], f32)
            dum_t = workp.tile([128, 1], f32)
            dsrc_t = workp.tile([128, 1], f32)

            # warm the exp table early (no DMA dependency: memset source)
            nc.vector.memset(dsrc_t[:], 0.0)
            nc.scalar.activation(dum_t[:], dsrc_t[:], Act.Exp)
            # ones matrix for column sums: memset instead of DMA
            nc.vector.memset(crb_t[:], 1.0)

            # yT static columns: [256,257]=1 (S), [258+g]=1 only in group g's
            # block (exports Sg through the vlad matmul)
            for e in range(4):
                for m in range(MT):
                    eng = nc.vector if e < 2 else nc.gpsimd
                    yT3 = yT_t[e][m].rearrange("p (g c) -> p g c", c=BW)
                    eng.memset(yT3[:, :, D:D + 2], 1.0)
                    eng.memset(yT3[:, :, D + 2:BW], 0.0)
                    for blk in range(2):
                        g = 2 * e + blk
                        eng.memset(
                            yT_t[e][m][:, blk * BW + D + 2 + g:
                                       blk * BW + D + 3 + g], 1.0)

            # ---------------- phase A: gk logits + exp ----------------
            for h, w3hv in enumerate((w3h0v, w3h1v)):
                for m in range(MT):
                    z_ps = ps.tile([128, 512], f32, name="z_ps", tag="zps",
                                   bufs=3)
                    for c in range(4):
                        nc.tensor.matmul(z_ps[:], xop(m, c),
                                         w3hv[:, 2 * c:2 * c + 2, :],
                                         start=(c == 0), stop=(c == 3),
                                         perf_mode=DR)
                    nc.scalar.activation(ex_t[m][:, h * 512:(h + 1) * 512],
                                         z_ps[:], Act.Exp, scale=EXS)

            # ---------------- phase B: gates (flipped layout) ---------------
            # zgT[g, m] = wgT^T x ; one 8-mm chain instead of 16 tiny matmuls
            zg_ps = ps.tile([128, 512], f32, name="zg_ps", tag="zps", bufs=3)
            for half, xhv in enumerate((xav, xbv)):
                for c in range(4):
                    nc.tensor.matmul(zg_ps[0:G, half * 256:(half + 1) * 256],
                                     wgtv[:, 2 * c:2 * c + 2, 0:G],
                                     xhv[:, 2 * c:2 * c + 2, :],
                                     start=(c == 0), stop=(c == 3),
                                     perf_mode=DR)
            nc.scalar.activation(egT_t[0:G, :], zg_ps[0:G, :], Act.Exp,
                                 scale=EXS)
            # sgT = egT/(egT + e^{-bg'})   (all in [8, M] layout)
            nc.vector.tensor_scalar_add(sgT_t[0:G, :], egT_t[0:G, :], ebgT_t)
            nc.vector.reciprocal(sgT_t[0:G, :], sgT_t[0:G, :])
            nc.vector.tensor_mul(sgT_t[0:G, :], egT_t[0:G, :], sgT_t[0:G, :])
            # transpose to sg[m, g] via PE (identity from cf)
            for m in range(MT):
                sgp_ps = ps.tile([128, G], f32, name="sgp", tag="gps", bufs=1)
                nc.tensor.transpose(sgp_ps[:], sgT_t[0:G, m * 128:(m + 1) * 128],
                                    i8_t)
                nc.vector.tensor_copy(out=sg_t[:, m * G:(m + 1) * G],
                                      in_=sgp_ps[:])

            # ---------------- phase C: softmax denominators ----------------
            for h in range(2):
                se_ps = ps.tile([128, 512], f32, name="se_ps", tag="zps", bufs=3)
                for m in range(MT):
                    nc.tensor.matmul(se_ps[:], crb_t[:],
                                     ex_t[m][:, h * 512:(h + 1) * 512],
                                     start=(m == 0), stop=(m == MT - 1))
                nc.vector.reciprocal_approx_fast(
                    out=ise_t[:, h * 512:(h + 1) * 512], in_=se_ps[:])

            # ---------------- wf = ex*sg*ise, all tiles up front (DVE) ------
            for g in range(G):
                gsl = slice(g * KC, (g + 1) * KC)
                for m in range(MT):
                    nc.vector.scalar_tensor_tensor(
                        out=wf_t[m][:, gsl], in0=ex_t[m][:, gsl],
                        scalar=sg_t[:, m * G + g:m * G + g + 1],
                        in1=ise_t[:, gsl], op0=Alu.mult, op1=Alu.mult)

            # ---------------- phase D: yT on Act, vlad trailing -------------
            vd_ps = ps.tile([128, 512], f32, name="vd_ps", tag="gps", bufs=1)
            for ec in range(4):
                for m in range(MT):
                    ry_ps = ps.tile([128, 512], f32, name="ry_ps", tag="yps",
                                    bufs=3)
                    w1hv = (w1av, w1bv)[ec // 2]
                    ecs = (ec % 2) * 512
                    for c in range(4):
                        nc.tensor.matmul(ry_ps[:], xop(m, c),
                                         w1hv[:, 2 * c:2 * c + 2, ecs:ecs + 512],
                                         start=(c == 0), stop=(c == 3),
                                         perf_mode=DR)
                    yT3 = yT_t[ec][m].rearrange("p (g c) -> p g c", c=BW)
                    nc.scalar.activation(
                        yT3[:, 0:2, 0:D],
                        ry_ps[:].rearrange("p (g c) -> p g c", c=D),
                        Act.Copy, scale=YTS)
                # vlad groups one ec late (their wf/yT are complete by now)
                if ec >= 1:
                    for g in (2 * ec - 2, 2 * ec - 1):
                        for m in range(MT):
                            nc.tensor.matmul(
                                vd_ps[:, 0:BW], wf_t[m][:, g * KC:(g + 1) * KC],
                                yT_t[g // 2][m][:, (g % 2) * BW:(g % 2 + 1) * BW],
                                start=(g == 0 and m == 0), stop=False)

            # ---------------- phase E: remaining vlad + centroid + out ------
            for g in (6, 7):
                for m in range(MT):
                    nc.tensor.matmul(vd_ps[:, 0:BW],
                                     wf_t[m][:, g * KC:(g + 1) * KC],
                                     yT_t[g // 2][m][:, (g % 2) * BW:(g % 2 + 1) * BW],
                                     start=False, stop=(g == 7 and m == MT - 1))
            vlad_t = workp.tile([128, D + G], f32)
            nc.vector.scalar_tensor_tensor(
                out=vlad_t[:, 0:D], in0=centn_t[:], scalar=vd_ps[:, D:D + 1],
                in1=vd_ps[:, 0:D], op0=Alu.mult, op1=Alu.add)
            nc.scalar.activation(vlad_t[:, D:D + G], vd_ps[:, D + 2:D + 2 + G],
                                 Act.Copy)
            nc.sync.dma_start(out=out_d[0:64, :], in_=vlad_t[0:64, :])
            nc.scalar.dma_start(out=out_d[64:128, :], in_=vlad_t[64:128, :])

    nc.compile()
    return nc


def _get_nc():
    if "nc" not in _cache:
        _cache["nc"] = _build_nc()
    return _cache["nc"]


def _pack(a):
    """[1024, C] -> [128, 8*C]: row p col (cs*C+j) = a[cs*128+p, j]."""
    c = a.shape[1]
    return np.ascontiguousarray(
        a.reshape(8, 128, c).transpose(1, 0, 2).reshape(128, 8 * c))


def kernel(x, W_inp, b_inp, W_g, b_g, W_gk, b_gk, centroids):
    from concourse.bass_utils import run_bass_kernel_spmd
    import ml_dtypes
    f8 = ml_dtypes.float8_e4m3

    nc = _get_nc()

    x = np.asarray(x, dtype=np.float32)
    X = x.reshape(8, 8, N, 64).transpose(0, 2, 1, 3).reshape(8, N, M)
    # exact per-token normalization on host
    X = X / np.maximum(np.linalg.norm(X, axis=1, keepdims=True), 1e-12)
    W1 = (np.asarray(W_inp, np.float32).T * SW1).astype(f8)
    W1a = _pack(W1[:, 0:1024])
    W1b = _pack(W1[:, 1024:2048])
    W3f = ((np.asarray(W_gk, np.float32) @ np.asarray(W_inp, np.float32)).T
           * SW3).astype(f8)
    W3h0 = _pack(W3f[:, 0:512])
    W3h1 = _pack(W3f[:, 512:1024])
    Wgp = np.zeros((N, 16), np.float32)
    Wgp[:, 0:G] = (np.asarray(W_g, np.float32)
                   @ np.asarray(W_inp, np.float32)).T * SW3
    Wgt = _pack(Wgp.astype(f8))
    bgp = (np.asarray(b_g, np.float32)
           + np.asarray(W_g, np.float32) @ np.asarray(b_inp, np.float32))
    cf = np.zeros((128, CFW), np.float32)
    cf[:, 0:D] = -np.asarray(centroids, np.float32)
    cf[0:8, 264:272] = np.eye(8, dtype=np.float32)
    cf[0:8, 272] = np.exp(-bgp)

    in_maps = []
    for b in range(8):
        Xs = (X[b] * SX).astype(f8)
        in_maps.append({"xa": _pack(Xs[:, 0:256]), "xb": _pack(Xs[:, 256:512]),
                        "w3h0": W3h0, "w3h1": W3h1, "wgt": Wgt,
                        "w1a": W1a, "w1b": W1b, "cf": cf})

    trace = os.environ.get("KERNEL_TRACE") == "1"
    r = run_bass_kernel_spmd(nc, in_maps, core_ids=list(range(8)), trace=trace)
    _cache["last_results"] = r
    binp_r = np.asarray(b_inp, np.float64).reshape(G, D)
    out = np.empty((8, KC * D), np.float32)
    for b in range(8):
        raw = r.results[b]["out"].astype(np.float64)        # [128, 264]
        # add back the Sg @ b_inp contribution (device computed y w/o b_inp)
        vlad = raw[:, 0:D] + raw[:, D:D + G] @ binp_r
        nrm = np.sqrt((vlad * vlad).sum(axis=1, keepdims=True))
        out[b] = (vlad / (np.maximum(nrm, 1e-12) * np.sqrt(128.0))
                  ).reshape(-1).astype(np.float32)
    return out


# revision 13
# speedup vs baseline: 1.0280x; 1.0280x over previous
"""NextVLAD Trainium2 kernel v7 — 8-way data-parallel over batch (1 sample/core).

Host prep: x is token-normalized on host (exact), so every device scale is a
compile-time constant; W3 = (W_gk@W_inp)^T*SW3 folds the fc projection for
the softmax path; WgT = (W_g@W_inp)^T*SW3 feeds a flipped gate matmul;
W1 = W_inp^T*SW1 feeds yT. Final L2 norms and the +b_inp contribution are
applied on host (the device exports per-group sums Sg via one-hot columns
in the vlad matmul).

v7 layout/schedule:
- every input host-packed to its exact [128, X] SBUF image -> each DMA is
  full-partition-row contiguous (big packets); one ring (sync), strict
  priority order: xa -> w3blk0 -> xb -> w3blk1 -> wgT -> cf -> w1a -> w1b.
- gate logits via a FLIPPED matmul (out [8, M]): one 8-mm chain instead of
  16 LDW-bound [*,8] matmuls; sigmoid computed in [8, M] layout, then 4 PE
  transposes (identity in cf) give sg[m, g] for the wf weights.
- yT = ry*YTS on the Act engine (Copy+scale from PSUM); wf = ex*sg*ise on
  DVE, all 32 tiles emitted right after the softmax denominators.

Per-core dataflow (sample b; M=512 tokens, N=1024, EN=2048, G=8, K=128, D=256):
  z[m,h512]  = x8^T W3-blk (fp8 DR)     ex = exp(z/16384)        bf16
  zgT[g,m]   = wgT^T x8 (fp8 DR)        egT = exp(zgT/16384)     f32
                                        sgT = egT/(egT+e^{-bg'}) DVE
                                        sg[m,g] = transpose(sgT) PE
  se[h]      = ones^T ex (bf16 mm)      ise = recip_approx(se)   f32
  rawY[m,e]  = x8^T W1-chunks (fp8 DR)  yT = rawY/128 (Act Copy) bf16
  wf[m,gk]   = ex * sg * ise (DVE)                               bf16
  vd[k,266]  = sum_{g,m} wf_g^T [yT_g | 1 1 | e_g]
               (col 256 = S[k], cols 258+g = Sg[k,g])
  out[k,:256]= vd - S*cent ; out[k,256:264] = Sg  (host: +Sg@binp, l2norm)
"""
import os
import numpy as np

N = 1024          # feature size
EN = 2048         # expanded features
G = 8             # groups
KC = 128          # clusters
D = 256           # per-group cluster dim
GK = G * KC       # 1024
BW = D + 10       # yT group block: 256 data | 2 ones | 8 one-hot = 266
M = 512           # tokens per sample
MT = 4            # m-tiles of 128
CFW = 280         # cf width: 256 cent | 8 spare | I8 (264:272) | ebgT (272)
SX = 8.0
SW1 = 16.0
SW3 = 2048.0
EXS = 1.0 / (SX * SW3)    # 1/16384
YTS = 1.0 / (SX * SW1)    # 1/128

_cache = {}


def _build_nc():
    import concourse.bacc as bacc
    import concourse.tile as tile
    from concourse import mybir

    f32 = mybir.dt.float32
    bf16 = mybir.dt.bfloat16
    fp8 = mybir.dt.float8e4
    Alu = mybir.AluOpType
    Act = mybir.ActivationFunctionType
    DR = mybir.MatmulPerfMode.DoubleRow

    nc = bacc.Bacc("TRN2", target_bir_lowering=False)
    # host-packed: row p, col (cs*W + j) holds source row cs*128+p, col j
    xa_d = nc.dram_tensor("xa", [128, 8 * 256], fp8, kind="ExternalInput")
    xb_d = nc.dram_tensor("xb", [128, 8 * 256], fp8, kind="ExternalInput")
    w3h0_d = nc.dram_tensor("w3h0", [128, 8 * 512], fp8, kind="ExternalInput")
    w3h1_d = nc.dram_tensor("w3h1", [128, 8 * 512], fp8, kind="ExternalInput")
    wgt_d = nc.dram_tensor("wgt", [128, 8 * 16], fp8, kind="ExternalInput")
    w1a_d = nc.dram_tensor("w1a", [128, 8 * 1024], fp8, kind="ExternalInput")
    w1b_d = nc.dram_tensor("w1b", [128, 8 * 1024], fp8, kind="ExternalInput")
    cf_d = nc.dram_tensor("cf", [128, CFW], f32, kind="ExternalInput")
    out_d = nc.dram_tensor("out", [KC, D + G], f32, kind="ExternalOutput")

    with tile.TileContext(nc) as tc:
        with tc.tile_pool(name="const", bufs=1) as constp, \
             tc.tile_pool(name="data", bufs=1) as datap, \
             tc.tile_pool(name="work", bufs=1) as workp, \
             tc.tile_pool(name="ps", bufs=1, space="PSUM") as ps:

            # ---------------- tiles ----------------
            cf_t = constp.tile([128, CFW], f32)
            crb_t = constp.tile([128, 128], bf16)
            centn_t = cf_t[:, 0:D]
            i8_t = cf_t[0:8, 264:272]
            ebgT_t = cf_t[0:8, 272:273]
            xa_t = datap.tile([128, 8 * 256], fp8)
            xav = xa_t.rearrange("p (cs m) -> p cs m", m=256)
            xb_t = datap.tile([128, 8 * 256], fp8)
            xbv = xb_t.rearrange("p (cs m) -> p cs m", m=256)
            w3h0_t = datap.tile([128, 8 * 512], fp8)
            w3h0v = w3h0_t.rearrange("p (cs j) -> p cs j", j=512)
            w3h1_t = datap.tile([128, 8 * 512], fp8)
            w3h1v = w3h1_t.rearrange("p (cs j) -> p cs j", j=512)
            wgt_t = datap.tile([128, 8 * 16], fp8)
            wgtv = wgt_t.rearrange("p (cs j) -> p cs j", j=16)
            w1a_t = datap.tile([128, 8 * 1024], fp8)
            w1av = w1a_t.rearrange("p (cs j) -> p cs j", j=1024)
            w1b_t = datap.tile([128, 8 * 1024], fp8)
            w1bv = w1b_t.rearrange("p (cs j) -> p cs j", j=1024)

            def xop(m, c):
                """lhsT for m-tile m, chunk c: [128, 2, 128]."""
                v = xav if m < 2 else xbv
                ms = (m % 2) * 128
                return v[:, 2 * c:2 * c + 2, ms:ms + 128]

            # ------------- input DMA: one ring (sync), strict priority order ------
            nc.sync.dma_start(out=xa_t[:], in_=xa_d[:])
            nc.sync.dma_start(out=w3h0_t[:], in_=w3h0_d[:])
            nc.sync.dma_start(out=xb_t[:], in_=xb_d[:])
            # tensors streaming DURING compute: 2KB runs (big bursts steal
            # SBUF ports from the PE rhs stream and inflate matmul time)
            nc.sync.dma_start(out=w3h1v[:, 0:4, :], in_=w3h1_d[:, 0:2048])
            nc.sync.dma_start(out=w3h1v[:, 4:8, :], in_=w3h1_d[:, 2048:4096])
            nc.sync.dma_start(out=wgt_t[:], in_=wgt_d[:])
            nc.sync.dma_start(out=cf_t[:], in_=cf_d[:])
            for q in range(4):
                nc.sync.dma_start(out=w1av[:, 2 * q:2 * q + 2, :],
                                  in_=w1a_d[:, q * 2048:(q + 1) * 2048])
            for q in range(4):
                nc.sync.dma_start(out=w1bv[:, 2 * q:2 * q + 2, :],
                                  in_=w1b_d[:, q * 2048:(q + 1) * 2048])

            # persistent work tiles
            ex_t = [workp.tile([128, GK], bf16, name=f"ex{m}") for m in range(MT)]
            wf_t = [workp.tile([128, GK], bf16, name=f"wf{m}") for m in range(MT)]
            yT_t = [[workp.tile([128, 2 * BW], bf16, name=f"yT{e}_{m}")
                     for m in range(MT)] for e in range(4)]
            ise_t = workp.tile([128, GK], f32)
            sg_t = workp.tile([128, 4 * G], f32)
            egT_t = workp.tile([128, M], f32)
            sgT_t = workp.tile([128, M], f32)
            dum_t = workp.tile([128, 1], f32)
            dsrc_t = workp.tile([128, 1], f32)

            # warm the exp table early (no DMA dependency: memset source)
            nc.vector.memset(dsrc_t[:], 0.0)
            nc.scalar.activation(dum_t[:], dsrc_t[:], Act.Exp)
            # ones matrix for column sums: memset instead of DMA
            nc.vector.memset(crb_t[:], 1.0)

            # yT static columns: [256,257]=1 (S), [258+g]=1 only in group g's
            # block (exports Sg through the vlad matmul)
            for e in range(4):
                for m in range(MT):
                    eng = nc.vector if e < 2 else nc.gpsimd
                    yT3 = yT_t[e][m].rearrange("p (g c) -> p g c", c=BW)
                    eng.memset(yT3[:, :, D:D + 2], 1.0)
                    eng.memset(yT3[:, :, D + 2:BW], 0.0)
                    for blk in range(2):
                        g = 2 * e + blk
                        eng.memset(
                            yT_t[e][m][:, blk * BW + D + 2 + g:
                                       blk * BW + D + 3 + g], 1.0)

            # ---------------- phase A: gk logits + exp ----------------
            for h, w3hv in enumerate((w3h0v, w3h1v)):
                for m in range(MT):
                    z_ps = ps.tile([128, 512], f32, name="z_ps", tag="zps",
                                   bufs=4)
                    for c in range(4):
                        nc.tensor.matmul(z_ps[:], xop(m, c),
                                         w3hv[:, 2 * c:2 * c + 2, :],
                                         start=(c == 0), stop=(c == 3),
                                         perf_mode=DR)
                    nc.scalar.activation(ex_t[m][:, h * 512:(h + 1) * 512],
                                         z_ps[:], Act.Exp, scale=EXS)

            # ---------------- phase B: gates (flipped layout) ---------------
            # zgT[g, m] = wgT^T x ; one 8-mm chain instead of 16 tiny matmuls
            zg_ps = ps.tile([128, 512], f32, name="zg_ps", tag="zps", bufs=4)
            for half, xhv in enumerate((xav, xbv)):
                for c in range(4):
                    nc.tensor.matmul(zg_ps[0:G, half * 256:(half + 1) * 256],
                                     wgtv[:, 2 * c:2 * c + 2, 0:G],
                                     xhv[:, 2 * c:2 * c + 2, :],
                                     start=(c == 0), stop=(c == 3),
                                     perf_mode=DR)
            nc.scalar.activation(egT_t[0:G, :], zg_ps[0:G, :], Act.Exp,
                                 scale=EXS)
            # sgT = egT/(egT + e^{-bg'})   (all in [8, M] layout)
            nc.vector.tensor_scalar_add(sgT_t[0:G, :], egT_t[0:G, :], ebgT_t)
            nc.vector.reciprocal(sgT_t[0:G, :], sgT_t[0:G, :])
            nc.vector.tensor_mul(sgT_t[0:G, :], egT_t[0:G, :], sgT_t[0:G, :])
            # transpose to sg[m, g] via PE (identity from cf)
            for m in range(MT):
                sgp_ps = ps.tile([128, G], f32, name="sgp", tag="gps", bufs=1)
                nc.tensor.transpose(sgp_ps[:], sgT_t[0:G, m * 128:(m + 1) * 128],
                                    i8_t)
                nc.vector.tensor_copy(out=sg_t[:, m * G:(m + 1) * G],
                                      in_=sgp_ps[:])

            # ---------------- phase C: softmax denominators ----------------
            for h in range(2):
                se_ps = ps.tile([128, 512], f32, name="se_ps", tag="zps", bufs=4)
                for m in range(MT):
                    nc.tensor.matmul(se_ps[:], crb_t[:],
                                     ex_t[m][:, h * 512:(h + 1) * 512],
                                     start=(m == 0), stop=(m == MT - 1))
                nc.vector.reciprocal_approx_fast(
                    out=ise_t[:, h * 512:(h + 1) * 512], in_=se_ps[:])

            # ---------------- wf = ex*sg*ise, all tiles up front (DVE) ------
            for g in range(G):
                gsl = slice(g * KC, (g + 1) * KC)
                for m in range(MT):
                    nc.vector.scalar_tensor_tensor(
                        out=wf_t[m][:, gsl], in0=ex_t[m][:, gsl],
                        scalar=sg_t[:, m * G + g:m * G + g + 1],
                        in1=ise_t[:, gsl], op0=Alu.mult, op1=Alu.mult)

            # ---------------- phase D: yT on Act, vlad trailing -------------
            vd_ps = ps.tile([128, 512], f32, name="vd_ps", tag="gps", bufs=1)
            for ec in range(4):
                for m in range(MT):
                    ry_ps = ps.tile([128, 512], f32, name="ry_ps", tag="yps",
                                    bufs=3)
                    w1hv = (w1av, w1bv)[ec // 2]
                    ecs = (ec % 2) * 512
                    for c in range(4):
                        nc.tensor.matmul(ry_ps[:], xop(m, c),
                                         w1hv[:, 2 * c:2 * c + 2, ecs:ecs + 512],
                                         start=(c == 0), stop=(c == 3),
                                         perf_mode=DR)
                    yT3 = yT_t[ec][m].rearrange("p (g c) -> p g c", c=BW)
                    nc.scalar.activation(
                        yT3[:, 0:2, 0:D],
                        ry_ps[:].rearrange("p (g c) -> p g c", c=D),
                        Act.Copy, scale=YTS)
                # vlad groups one ec late (their wf/yT are complete by now)
                if ec >= 1:
                    for g in (2 * ec - 2, 2 * ec - 1):
                        for m in range(MT):
                            nc.tensor.matmul(
                                vd_ps[:, 0:BW], wf_t[m][:, g * KC:(g + 1) * KC],
                                yT_t[g // 2][m][:, (g % 2) * BW:(g % 2 + 1) * BW],
                                start=(g == 0 and m == 0), stop=False)

            # ---------------- phase E: remaining vlad + centroid + out ------
            for g in (6, 7):
                for m in range(MT):
                    nc.tensor.matmul(vd_ps[:, 0:BW],
                                     wf_t[m][:, g * KC:(g + 1) * KC],
                                     yT_t[g // 2][m][:, (g % 2) * BW:(g % 2 + 1) * BW],
                                     start=False, stop=(g == 7 and m == MT - 1))
            vlad_t = workp.tile([128, D + G], f32)
            nc.vector.scalar_tensor_tensor(
                out=vlad_t[:, 0:D], in0=centn_t[:], scalar=vd_ps[:, D:D + 1],
                in1=vd_ps[:, 0:D], op0=Alu.mult, op1=Alu.add)
            nc.scalar.activation(vlad_t[:, D:D + G], vd_ps[:, D + 2:D + 2 + G],
                                 Act.Copy)
            nc.sync.dma_start(out=out_d[0:64, :], in_=vlad_t[0:64, :])
            nc.scalar.dma_start(out=out_d[64:128, :], in_=vlad_t[64:128, :])

    nc.compile()
    return nc


def _get_nc():
    if "nc" not in _cache:
        _cache["nc"] = _build_nc()
    return _cache["nc"]


def _pack(a):
    """[1024, C] -> [128, 8*C]: row p col (cs*C+j) = a[cs*128+p, j]."""
    c = a.shape[1]
    return np.ascontiguousarray(
        a.reshape(8, 128, c).transpose(1, 0, 2).reshape(128, 8 * c))


def kernel(x, W_inp, b_inp, W_g, b_g, W_gk, b_gk, centroids):
    from concourse.bass_utils import run_bass_kernel_spmd
    import ml_dtypes
    f8 = ml_dtypes.float8_e4m3

    nc = _get_nc()

    x = np.asarray(x, dtype=np.float32)
    X = x.reshape(8, 8, N, 64).transpose(0, 2, 1, 3).reshape(8, N, M)
    # exact per-token normalization on host
    X = X / np.maximum(np.linalg.norm(X, axis=1, keepdims=True), 1e-12)
    W1 = (np.asarray(W_inp, np.float32).T * SW1).astype(f8)
    W1a = _pack(W1[:, 0:1024])
    W1b = _pack(W1[:, 1024:2048])
    W3f = ((np.asarray(W_gk, np.float32) @ np.asarray(W_inp, np.float32)).T
           * SW3).astype(f8)
    W3h0 = _pack(W3f[:, 0:512])
    W3h1 = _pack(W3f[:, 512:1024])
    Wgp = np.zeros((N, 16), np.float32)
    Wgp[:, 0:G] = (np.asarray(W_g, np.float32)
                   @ np.asarray(W_inp, np.float32)).T * SW3
    Wgt = _pack(Wgp.astype(f8))
    bgp = (np.asarray(b_g, np.float32)
           + np.asarray(W_g, np.float32) @ np.asarray(b_inp, np.float32))
    cf = np.zeros((128, CFW), np.float32)
    cf[:, 0:D] = -np.asarray(centroids, np.float32)
    cf[0:8, 264:272] = np.eye(8, dtype=np.float32)
    cf[0:8, 272] = np.exp(-bgp)

    in_maps = []
    for b in range(8):
        Xs = (X[b] * SX).astype(f8)
        in_maps.append({"xa": _pack(Xs[:, 0:256]), "xb": _pack(Xs[:, 256:512]),
                        "w3h0": W3h0, "w3h1": W3h1, "wgt": Wgt,
                        "w1a": W1a, "w1b": W1b, "cf": cf})

    trace = os.environ.get("KERNEL_TRACE") == "1"
    r = run_bass_kernel_spmd(nc, in_maps, core_ids=list(range(8)), trace=trace)
    _cache["last_results"] = r
    binp_r = np.asarray(b_inp, np.float64).reshape(G, D)
    out = np.empty((8, KC * D), np.float32)
    for b in range(8):
        raw = r.results[b]["out"].astype(np.float64)        # [128, 264]
        # add back the Sg @ b_inp contribution (device computed y w/o b_inp)
        vlad = raw[:, 0:D] + raw[:, D:D + G] @ binp_r
        nrm = np.sqrt((vlad * vlad).sum(axis=1, keepdims=True))
        out[b] = (vlad / (np.maximum(nrm, 1e-12) * np.sqrt(128.0))
                  ).reshape(-1).astype(np.float32)
    return out


# revision 14
# speedup vs baseline: 1.0566x; 1.0278x over previous
"""NextVLAD Trainium2 kernel v9 — 8-way data-parallel over batch (1 sample/core).

Host prep: x is token-normalized on host (exact), so every device scale is a
compile-time constant; W3 = [(W_gk@W_inp)^T | (W_g@W_inp)^T]*SW3 folds the
fc and gate projections for the softmax path; W1 = W_inp^T*SW1 feeds yT.
Final L2 norms and the +b_inp contribution to the VLAD sum are applied on
host (the device exports per-group sums Sg via one-hot columns in the vlad
matmul, so b_inp never has to be broadcast on-device).

v9 layout/schedule (= v6b structure, measured A-phase 175 ns/mm):
- inputs host-packed to [128, X] SBUF images; one ring (sync), strict
  priority order: x(m<256) -> w3(blk0) -> x(m>=256) -> w3(blk1,2) -> cf ->
  w1a -> w1b; w1 streams in 2KB-run pieces (big bursts steal SBUF ports
  from the PE rhs stream and inflate matmul time).
- gate logits folded into the z matmul: 1040-wide W3 (1024 gk + 8 gate +
  pad) processed as 3 col-blocks of 344 -> no separate gate matmuls.
- yT = ry*YTS on the Act engine (Copy+scale from PSUM); wf = ex*sg*ise on
  DVE, all 32 tiles emitted right after the softmax denominators.
- tail: final centroid STT split into column halves so the first output
  DMA overlaps the second half.

Per-core dataflow (sample b; M=512 tokens, N=1024, EN=2048, G=8, K=128, D=256):
  z[m,b344]  = x8^T W3-blk (fp8 DR)     ex = exp(z/16384)       bf16
  eg[m,g]    = exp(z_gate/16384) (f32)  sg = eg/(eg+e^{-bg'})   DVE
  se[blk]    = ones^T ex (bf16 mm)      ise = recip_approx(se)  f32
  rawY[m,e]  = x8^T W1-chunks (fp8 DR)  yT = rawY/128 (Act Copy) bf16
  wf[m,gk]   = ex * sg * ise (DVE)                              bf16
  vd[k,266]  = sum_{g,m} wf_g^T [yT_g | 1 1 | e_g]
               (col 256 = S[k], cols 258+g = Sg[k,g])
  out[k,:256]= vd - S*cent ; out[k,256:264] = Sg   (host: +Sg@binp, l2norm)
"""
import os
import numpy as np

N = 1024          # feature size
EN = 2048         # expanded features
G = 8             # groups
KC = 128          # clusters
D = 256           # per-group cluster dim
GK = G * KC       # 1024
BW = D + 10       # yT group block: 256 data | 2 ones | 8 one-hot = 266
W3W = 1040        # 1024 gk + 8 gates + 8 pad (row stride must be %16)
ZB = 344          # z col-block width (3 blocks cover 1032)
M = 512           # tokens per sample
MT = 4            # m-tiles of 128
SX = 8.0
SW1 = 16.0
SW3 = 2048.0
EXS = 1.0 / (SX * SW3)    # 1/16384
YTS = 1.0 / (SX * SW1)    # 1/128

_cache = {}


def _build_nc():
    import concourse.bacc as bacc
    import concourse.tile as tile
    from concourse import mybir

    f32 = mybir.dt.float32
    bf16 = mybir.dt.bfloat16
    fp8 = mybir.dt.float8e4
    Alu = mybir.AluOpType
    Act = mybir.ActivationFunctionType
    DR = mybir.MatmulPerfMode.DoubleRow

    nc = bacc.Bacc("TRN2", target_bir_lowering=False)
    # host-packed: row p, col (cs*W + j) holds source row cs*128+p, col j
    x0_d = nc.dram_tensor("x0", [128, 8 * 256], fp8, kind="ExternalInput")
    x1_d = nc.dram_tensor("x1", [128, 8 * 256], fp8, kind="ExternalInput")
    w3p0_d = nc.dram_tensor("w3p0", [128, 8 * ZB], fp8, kind="ExternalInput")
    w3p12_d = nc.dram_tensor("w3p12", [128, 8 * (W3W - ZB)], fp8,
                             kind="ExternalInput")
    w1a_d = nc.dram_tensor("w1a", [128, 8 * 1024], fp8, kind="ExternalInput")
    w1b_d = nc.dram_tensor("w1b", [128, 8 * 1024], fp8, kind="ExternalInput")
    cf_d = nc.dram_tensor("cf", [128, 268], f32, kind="ExternalInput")
    out_d = nc.dram_tensor("out", [KC, D + G], f32, kind="ExternalOutput")

    with tile.TileContext(nc) as tc:
        with tc.tile_pool(name="const", bufs=1) as constp, \
             tc.tile_pool(name="data", bufs=1) as datap, \
             tc.tile_pool(name="work", bufs=1) as workp, \
             tc.tile_pool(name="ps", bufs=1, space="PSUM") as ps:

            # ---------------- tiles ----------------
            cf_t = constp.tile([128, 268], f32)
            crb_t = constp.tile([128, 128], bf16)
            centn_t = cf_t[:, 0:D]
            ebg_t = cf_t[:, 256:264]
            xp_t = datap.tile([128, 8 * M], fp8)
            xv = xp_t.rearrange("p (cs m) -> p cs m", m=M)
            w3_t = datap.tile([128, 8 * W3W], fp8)
            w3v = w3_t.rearrange("p (cs j) -> p cs j", j=W3W)
            w1a_t = datap.tile([128, 8 * 1024], fp8)
            w1av = w1a_t.rearrange("p (cs j) -> p cs j", j=1024)
            w1b_t = datap.tile([128, 8 * 1024], fp8)
            w1bv = w1b_t.rearrange("p (cs j) -> p cs j", j=1024)

            # ------------- input DMA: one ring (sync), strict priority order ------
            nc.sync.dma_start(out=xv[:, :, 0:256], in_=x0_d[:])
            nc.sync.dma_start(out=w3v[:, :, 0:ZB], in_=w3p0_d[:])
            nc.sync.dma_start(out=xv[:, :, 256:512], in_=x1_d[:])
            nc.sync.dma_start(out=w3v[:, :, ZB:W3W], in_=w3p12_d[:])
            nc.sync.dma_start(out=cf_t[:], in_=cf_d[:])
            for q in range(4):
                nc.sync.dma_start(out=w1av[:, 2 * q:2 * q + 2, :],
                                  in_=w1a_d[:, q * 2048:(q + 1) * 2048])
            for q in range(4):
                nc.sync.dma_start(out=w1bv[:, 2 * q:2 * q + 2, :],
                                  in_=w1b_d[:, q * 2048:(q + 1) * 2048])

            # persistent work tiles
            ex_t = [workp.tile([128, 3 * ZB], bf16, name=f"ex{m}")
                    for m in range(MT)]
            wf_t = [workp.tile([128, GK], bf16, name=f"wf{m}") for m in range(MT)]
            yT_t = [[workp.tile([128, 2 * BW], bf16, name=f"yT{e}_{m}")
                     for m in range(MT)] for e in range(4)]
            ise_t = workp.tile([128, 3 * ZB], f32)
            sg_t = workp.tile([128, 4 * G], f32)
            eg_t = workp.tile([128, 4 * G], f32)
            dum_t = workp.tile([128, 1], f32)
            dsrc_t = workp.tile([128, 1], f32)

            # warm the exp table early (no DMA dependency: memset source)
            nc.vector.memset(dsrc_t[:], 0.0)
            nc.scalar.activation(dum_t[:], dsrc_t[:], Act.Exp)
            # ones matrix for column sums: memset instead of DMA
            nc.vector.memset(crb_t[:], 1.0)

            # yT static columns: [256,257]=1 (S), [258+g]=1 only in group g's
            # block (exports Sg through the vlad matmul)
            for e in range(4):
                for m in range(MT):
                    eng = nc.vector if e < 2 else nc.gpsimd
                    yT3 = yT_t[e][m].rearrange("p (g c) -> p g c", c=BW)
                    eng.memset(yT3[:, :, D:D + 2], 1.0)
                    eng.memset(yT3[:, :, D + 2:BW], 0.0)
                    for blk in range(2):
                        g = 2 * e + blk
                        eng.memset(
                            yT_t[e][m][:, blk * BW + D + 2 + g:
                                       blk * BW + D + 3 + g], 1.0)

            # ---------------- phase A: gk+gate logits + exp ----------------
            # 3 col-blocks of 344 over the 1040-wide fused W3
            for blk in range(3):
                csl = slice(blk * ZB, (blk + 1) * ZB)
                for m in range(MT):
                    msl = slice(m * 128, (m + 1) * 128)
                    z_ps = ps.tile([128, 512], f32, name="z_ps", tag="zps",
                                   bufs=4)
                    for c in range(4):
                        nc.tensor.matmul(z_ps[:, 0:ZB],
                                         xv[:, 2 * c:2 * c + 2, msl],
                                         w3v[:, 2 * c:2 * c + 2, csl],
                                         start=(c == 0), stop=(c == 3),
                                         perf_mode=DR)
                    nc.scalar.activation(ex_t[m][:, csl], z_ps[:, 0:ZB],
                                         Act.Exp, scale=EXS)
                    if blk == 2:
                        # gate logits live in cols 1024:1032 = blk2 336:344
                        nc.scalar.activation(eg_t[:, m * G:(m + 1) * G],
                                             z_ps[:, ZB - 8:ZB],
                                             Act.Exp, scale=EXS)

            # ---------------- phase B: gates sg = eg/(eg+e^{-bg'}) (DVE) ----
            for m in range(MT):
                gs = slice(m * G, (m + 1) * G)
                wg = workp.tile([128, G], f32, name="wg", bufs=2)
                nc.vector.tensor_add(wg[:], eg_t[:, gs], ebg_t)
                rw = workp.tile([128, G], f32, name="rw", bufs=2)
                nc.vector.reciprocal(rw[:], wg[:])
                nc.vector.tensor_mul(sg_t[:, gs], eg_t[:, gs], rw[:])

            # ---------------- phase C: softmax denominators ----------------
            for blk in range(3):
                csl = slice(blk * ZB, (blk + 1) * ZB)
                se_ps = ps.tile([128, 512], f32, name="se_ps", tag="zps", bufs=4)
                for m in range(MT):
                    nc.tensor.matmul(se_ps[:, 0:ZB], crb_t[:],
                                     ex_t[m][:, csl],
                                     start=(m == 0), stop=(m == MT - 1))
                nc.vector.reciprocal_approx_fast(
                    out=ise_t[:, csl], in_=se_ps[:, 0:ZB])

            # ---------------- wf = ex*sg*ise, all tiles up front (DVE) ------
            for g in range(G):
                gsl = slice(g * KC, (g + 1) * KC)
                for m in range(MT):
                    nc.vector.scalar_tensor_tensor(
                        out=wf_t[m][:, gsl], in0=ex_t[m][:, gsl],
                        scalar=sg_t[:, m * G + g:m * G + g + 1],
                        in1=ise_t[:, gsl], op0=Alu.mult, op1=Alu.mult)

            # ---------------- phase D: yT on Act, vlad trailing -------------
            vd_ps = ps.tile([128, 512], f32, name="vd_ps", tag="gps", bufs=1)
            for ec in range(4):
                for m in range(MT):
                    msl = slice(m * 128, (m + 1) * 128)
                    ry_ps = ps.tile([128, 512], f32, name="ry_ps", tag="yps",
                                    bufs=3)
                    w1hv = (w1av, w1bv)[ec // 2]
                    ecs = (ec % 2) * 512
                    for c in range(4):
                        nc.tensor.matmul(ry_ps[:], xv[:, 2 * c:2 * c + 2, msl],
                                         w1hv[:, 2 * c:2 * c + 2, ecs:ecs + 512],
                                         start=(c == 0), stop=(c == 3),
                                         perf_mode=DR)
                    yT3 = yT_t[ec][m].rearrange("p (g c) -> p g c", c=BW)
                    nc.scalar.activation(
                        yT3[:, 0:2, 0:D],
                        ry_ps[:].rearrange("p (g c) -> p g c", c=D),
                        Act.Copy, scale=YTS)
                # vlad groups one ec late (their wf/yT are complete by now)
                if ec >= 1:
                    for g in (2 * ec - 2, 2 * ec - 1):
                        for m in range(MT):
                            nc.tensor.matmul(
                                vd_ps[:, 0:BW], wf_t[m][:, g * KC:(g + 1) * KC],
                                yT_t[g // 2][m][:, (g % 2) * BW:(g % 2 + 1) * BW],
                                start=(g == 0 and m == 0), stop=False)

            # ---------------- phase E: remaining vlad + centroid + out ------
            for g in (6, 7):
                for m in range(MT):
                    nc.tensor.matmul(vd_ps[:, 0:BW],
                                     wf_t[m][:, g * KC:(g + 1) * KC],
                                     yT_t[g // 2][m][:, (g % 2) * BW:(g % 2 + 1) * BW],
                                     start=False, stop=(g == 7 and m == MT - 1))
            vlad_t = workp.tile([128, D + G], f32)
            # column-split tail: first output DMA overlaps the second STT half
            nc.vector.scalar_tensor_tensor(
                out=vlad_t[:, 0:128], in0=centn_t[:, 0:128],
                scalar=vd_ps[:, D:D + 1],
                in1=vd_ps[:, 0:128], op0=Alu.mult, op1=Alu.add)
            nc.sync.dma_start(out=out_d[:, 0:128], in_=vlad_t[:, 0:128])
            nc.vector.scalar_tensor_tensor(
                out=vlad_t[:, 128:D], in0=centn_t[:, 128:D],
                scalar=vd_ps[:, D:D + 1],
                in1=vd_ps[:, 128:D], op0=Alu.mult, op1=Alu.add)
            nc.scalar.activation(vlad_t[:, D:D + G], vd_ps[:, D + 2:D + 2 + G],
                                 Act.Copy)
            nc.scalar.dma_start(out=out_d[:, 128:D + G],
                                in_=vlad_t[:, 128:D + G])

    nc.compile()
    return nc


def _get_nc():
    if "nc" not in _cache:
        _cache["nc"] = _build_nc()
    return _cache["nc"]


def _pack(a):
    """[1024, C] -> [128, 8*C]: row p col (cs*C+j) = a[cs*128+p, j]."""
    c = a.shape[1]
    return np.ascontiguousarray(
        a.reshape(8, 128, c).transpose(1, 0, 2).reshape(128, 8 * c))


def kernel(x, W_inp, b_inp, W_g, b_g, W_gk, b_gk, centroids):
    from concourse.bass_utils import run_bass_kernel_spmd
    import ml_dtypes
    f8 = ml_dtypes.float8_e4m3

    nc = _get_nc()

    x = np.asarray(x, dtype=np.float32)
    X = x.reshape(8, 8, N, 64).transpose(0, 2, 1, 3).reshape(8, N, M)
    # exact per-token normalization on host
    X = X / np.maximum(np.linalg.norm(X, axis=1, keepdims=True), 1e-12)
    W1 = (np.asarray(W_inp, np.float32).T * SW1).astype(f8)
    W1a = _pack(W1[:, 0:1024])
    W1b = _pack(W1[:, 1024:2048])
    W3f = (np.asarray(W_gk, np.float32) @ np.asarray(W_inp, np.float32)).T
    WgT = (np.asarray(W_g, np.float32) @ np.asarray(W_inp, np.float32)).T
    W3 = np.zeros((N, W3W), np.float32)
    W3[:, 0:GK] = W3f * SW3
    W3[:, GK:GK + G] = WgT * SW3
    W3 = W3.astype(f8)
    W3p0 = _pack(W3[:, 0:ZB])
    W3p12 = _pack(W3[:, ZB:W3W])
    bgp = (np.asarray(b_g, np.float32)
           + np.asarray(W_g, np.float32) @ np.asarray(b_inp, np.float32))
    cf = np.zeros((128, 268), np.float32)
    cf[:, 0:D] = -np.asarray(centroids, np.float32)
    cf[:, 256:264] = np.exp(-bgp)[None, :]

    in_maps = []
    for b in range(8):
        Xs = (X[b] * SX).astype(f8)
        in_maps.append({"x0": _pack(Xs[:, 0:256]), "x1": _pack(Xs[:, 256:512]),
                        "w3p0": W3p0, "w3p12": W3p12,
                        "w1a": W1a, "w1b": W1b, "cf": cf})

    trace = os.environ.get("KERNEL_TRACE") == "1"
    r = run_bass_kernel_spmd(nc, in_maps, core_ids=list(range(8)), trace=trace)
    _cache["last_results"] = r
    binp_r = np.asarray(b_inp, np.float64).reshape(G, D)
    out = np.empty((8, KC * D), np.float32)
    for b in range(8):
        raw = r.results[b]["out"].astype(np.float64)        # [128, 264]
        # add back the Sg @ b_inp contribution (device computed y w/o b_inp)
        vlad = raw[:, 0:D] + raw[:, D:D + G] @ binp_r
        nrm = np.sqrt((vlad * vlad).sum(axis=1, keepdims=True))
        out[b] = (vlad / (np.maximum(nrm, 1e-12) * np.sqrt(128.0))
                  ).reshape(-1).astype(np.float32)
    return out
